# revision 59
# baseline (speedup 1.0000x reference)
"""Trainium2 Bass kernel: 7x7 local window attention (ConvNDAttention).

Input  X: [4, 64, 64, 256] fp32 (channel-last).
Output:   [4, 58, 58, 256] fp32.

For each output position (b, r, w): 7x7 input window rows r..r+6, cols
w..w+6; query = center cell (r+3, w+3); keys/values = the other 48 cells.
out = softmax(q . K / 16) @ K.

Sharding: 8 cores = 4 batches x 2 row-halves (30 output rows each, 2-row
overlap).  Per core, 18 tiles of 10x10 queries; each tile's keys are a 16x16
input patch (256 keys, 2 chunks of 128).

Design notes (cost-model driven; baseline 48.2us -> ~20.3us per core):
  - Each DMA costs ~625ns on the serialized HWDGE queue + ~650ns issue +
    ~900ns completion-sem, and all transfers serialize on the DMA engines
    at ~360 B/ns, so the whole input ships as ONE packed DRAM tensor
    ("blob", 4.2MB bf16) split into 7 paired per-panel DMAs: cut c carries
    vs(panel c-1) + xp(panel c), so compute on panel c starts as soon as
    its cut lands while panel c-1's AV data arrives just in time.
  - The window-validity mask is folded into the scores PSUM accumulation
    as a 5th matmul per (tile, key-chunk): PSUM[k,q] += cb[q,j,k] with
    rhs = I_100 (contraction over q).  Invalid pairs get -1e4 (exp
    underflows to exactly 0), valid pairs exact +0 -- no separate
    mask multiply, no extra cross-engine hop, no precision impact.
  - Two tiles' score PSUM share one 2KB bank ([128,2,2,100] fp32), so ONE
    ACT exp instruction converts both (amortizes ~143ns PSUM access cost).
  - PE queue is in-order: scores of pair m+1 are emitted before AV of pair
    m, so the PE never head-of-line blocks waiting for exp.
  - Normalize (out = AV[:, :256] * 1/AV[:, 256]) runs mostly on DVE, every
    4th tile on ACT (Pool/GPSIMD cannot read PSUM on real hw).
  - Output accumulates in SBUF bf16 and leaves in 4 chunks: early chunks
    via Pool/SWDGE (latency-tolerant), the tiny last chunk via SP/HWDGE.

Host-side layouts per core (bf16), packed into blob [128, 16520]:
  cb [100, 356]        mask-bias weights (-1e4/0) + identity I_100.
  xp [128, 2, 576]/panel  channel-major column panels (panel ti covers
      input cols W0S[ti]..+15, all 36 rows; free index s = row*16 + col;
      chunk k = channels 128k..+128 on partitions).
  vs [128, 3, 2, 257]/panel  spatial-major values + ones column per tile
      (partition = key-in-chunk = (row%8)*16+col, chunk j = key rows
      r0+8j..+8; the ones column turns AV col 256 into softmax row sums).
  out [100, 18, 256]   bf16; host scatters tiles into the full output.
"""

import numpy as np
import ml_dtypes

import concourse.bass as bass
import concourse.bacc as bacc
import concourse.mybir as mybir
import concourse.tile as tile

BF16 = ml_dtypes.bfloat16

# ---------------- geometry (hardcoded for X [4,64,64,256]) ----------------
B, H, W, C = 4, 64, 64, 256
HO, WO = H - 6, W - 6          # 58 x 58 output
N_CORES = 8
SH_ROWS_IN = 36                # input rows per shard
SH_ROWS_OUT = 30               # output rows per shard
R0S = [0, 10, 20]              # tile row origins (shard-local output rows)
W0S = [0, 10, 20, 30, 40, 48]  # tile col origins
QT = 10                        # query tile side
KT = 16                        # key patch side
NQ = QT * QT                   # 100 queries per tile
NT = len(W0S) * len(R0S)       # 18 tiles
PAN = SH_ROWS_IN * KT          # 576 panel spatial positions
BLOB_CB = 2 * 128 + NQ         # 356: bias weights + identity
PANEL_ELEMS = 2 * PAN + 3 * 2 * (C + 1)     # 2694: xp panel + 3 vs tiles
BLOB_TOT = BLOB_CB + len(W0S) * PANEL_ELEMS  # 16520 bf16 elems/partition
# processing order (tile 2 deferred to the end; see _build_bass); the
# device output tensor is indexed by POSITION in this order
TILE_ORDER = list(range(18))


def _build_bias_consts():
    """cb [100, 2*128 + 100] bf16: score-bias weights + identity.

    cb[q, j*128+k] = -1e4 where key (k, chunk j) is invalid for query q
    (exp underflows to exactly 0), 0 where valid (exact no-op).
    cb[q, 256:356] = I_100.  Added into the scores PSUM group via
    matmul(lhsT=cb[:, j], rhs=I): out[k, q] += cb[q, j*128+k] * I[q, q].
    """
    p = np.arange(128)
    kh = p[None, None, :] // KT + 8 * np.arange(2)[None, :, None]  # [1,2,128]
    kw = (p % KT)[None, None, :]
    q = np.arange(NQ)[:, None, None]
    qh, qw = q // QT, q % QT
    dy, dx = kh - qh, kw - qw
    valid = ((dy >= 0) & (dy <= 6) & (dx >= 0) & (dx <= 6)
             & ~((dy == 3) & (dx == 3)))                  # [100, 2, 128]
    cb = np.zeros((NQ, 2 * 128 + NQ), dtype=np.float32)
    cb[:, : 2 * 128] = np.where(valid, 0.0, -1e4).reshape(NQ, 256)
    cb[:, 2 * 128 :] = np.eye(NQ)
    return cb.astype(BF16)


_CBIAS = _build_bias_consts()

_NC_CACHE = None


def _build_bass():
    global _NC_CACHE
    if _NC_CACHE is not None:
        return _NC_CACHE
    nc = bacc.Bacc("TRN2")
    dt = mybir.dt

    blob = nc.dram_tensor("blob", [128, BLOB_TOT], dt.bfloat16,
                          kind="ExternalInput")
    out = nc.dram_tensor("out", [NQ, NT, C], dt.bfloat16,
                         kind="ExternalOutput")

    with tile.TileContext(nc) as tc:
        with (
            tc.tile_pool(name="const", bufs=1) as const_pool,
            tc.tile_pool(name="ework", bufs=6) as e_pool,
            tc.tile_pool(name="rwork", bufs=6) as r_pool,
            tc.tile_pool(name="owork", bufs=2) as o_pool,
            tc.tile_pool(name="ps_s", bufs=2, space="PSUM") as ps_s,
            tc.tile_pool(name="ps_av", bufs=6, space="PSUM") as ps_av,
            # ^ ps_s tiles hold TWO tiles' scores in one 1600B bank so one
            #   ACT exp instruction covers both (fixed access cost amortized)
        ):
            blob_sb = const_pool.tile([128, BLOB_TOT], dt.bfloat16, tag="blob")
            # paired per-panel loads: cut c delivers vs(panel c-1) + xp(panel
            # c) so S(panel c) / AV(panel c-1) start as soon as it lands
            cuts = ([0, BLOB_CB + 2 * PAN]
                    + [BLOB_CB + 2 * PAN + (t + 1) * PANEL_ELEMS
                       for t in range(len(W0S) - 1)]
                    + [BLOB_TOT])
            for a, b in zip(cuts[:-1], cuts[1:]):
                nc.sync.dma_start(out=blob_sb[:, a:b], in_=blob[:, a:b])

            cb_v = blob_sb[:NQ, : 2 * 128].rearrange("q (a k) -> q a k", k=128)
            iden = blob_sb[:NQ, 2 * 128 : BLOB_CB]

            def xp_pan(ti):
                o = BLOB_CB + ti * PANEL_ELEMS
                return blob_sb[:, o : o + 2 * PAN].rearrange(
                    "p (k s) -> p k s", s=PAN)

            def vs_pan(ti):
                o = BLOB_CB + ti * PANEL_ELEMS + 2 * PAN
                return blob_sb[:, o : o + 6 * (C + 1)].rearrange(
                    "p (r j c) -> p r j c", j=2, c=C + 1)

            o_all = o_pool.tile([NQ, NT, C], dt.bfloat16, tag="oall")
            # output chunks aligned to input-panel arrival; the last chunks
            # are tiny so the final DMA latency is minimal
            OCUTS = [0, 6, 12, 15, NT]

            # processing order: one early-panel tile (its data lands in the
            # first DMAs) is deferred to the END, so the kernel's last
            # AV->norm->out chain never waits on the final input DMA and
            # overlaps panel 5's own tail instead.
            tiles_nat = [(ti, w0, ri, r0)
                         for ti, w0 in enumerate(W0S)
                         for ri, r0 in enumerate(R0S)]
            tiles = [tiles_nat[t] for t in TILE_ORDER]

            def emit_scores(t, st):
                # writes scores into half t%2 of the pair tile st
                ti, w0, ri, r0 = tiles[t]
                xpt = xp_pan(ti)
                pan = [xpt[:, k, :] for k in range(2)]
                pan_v = [p.rearrange("p (h w) -> p h w", w=KT) for p in pan]
                for j in range(2):
                    for k in range(2):
                        keys = pan[k][:, (r0 + 8 * j) * KT :
                                      (r0 + 8 * j) * KT + 128]
                        qrys = pan_v[k][:, r0 + 3 : r0 + 3 + QT, 3 : 3 + QT]
                        nc.tensor.matmul(st[:, t % 2, j, :], lhsT=keys,
                                         rhs=qrys, start=(k == 0), stop=False)
                    # window-validity bias: st[k, q] += cb[q, j, k] (-1e4 on
                    # invalid pairs -> exp underflows to 0; exact 0 on valid)
                    nc.tensor.matmul(st[:, t % 2, j, :], lhsT=cb_v[:, j, :],
                                     rhs=iden, start=False, stop=True)

            def emit_pair_scores(m):
                st = ps_s.tile([128, 2, 2, NQ], dt.float32, tag="st")
                emit_scores(2 * m, st)
                if 2 * m + 1 < NT:
                    emit_scores(2 * m + 1, st)
                return st

            def emit_exp(m, st):
                # E = exp(S/16) -> bf16 SBUF, both tiles of the pair at once
                e = e_pool.tile([128, 2, 2, NQ], dt.bfloat16, tag="e")
                nc.scalar.activation(e[:, :, :, :], st[:, :, :, :],
                                     mybir.ActivationFunctionType.Exp,
                                     scale=1.0 / 16.0)
                return e

            def emit_av(t, e):
                ti, w0, ri, r0 = tiles[t]
                av = ps_av.tile([NQ, C + 1], dt.float32, tag="av")
                for j in range(2):
                    nc.tensor.matmul(av[:, :], lhsT=e[:, t % 2, j, :],
                                     rhs=vs_pan(ti)[:, ri, j, :],
                                     start=(j == 0), stop=(j == 1))
                return av

            def emit_norm(t, av):
                rinv = r_pool.tile([NQ, 1], dt.float32, tag="rinv")
                nc.vector.reciprocal(rinv[:, :], av[:, C : C + 1])
                dst = o_all[:, t, :]
                # mostly DVE; ACT (exp-laden) takes every 4th to balance
                # (Pool/GPSIMD cannot read PSUM on hw); t=16 on ACT so the
                # tail norms run on two engines in parallel
                if t % 4 == 2 or t == 16:
                    nc.scalar.mul(dst, av[:, 0:C], rinv[:, :])
                else:
                    nc.vector.tensor_scalar_mul(dst, av[:, 0:C], rinv[:, :])

            def emit_outs(t):
                for ci, (a, b) in enumerate(zip(OCUTS[:-1], OCUTS[1:])):
                    if t == b - 1:   # positions a..b-1 normalized by now
                        # big early chunks go via Pool/SWDGE (latency-
                        # tolerant); tail chunks via SP's faster HWDGE path
                        eng = nc.sync if b >= 15 else nc.gpsimd
                        eng.dma_start(out=out[:, a:b, :],
                                      in_=o_all[:, a:b, :])

            # software pipeline in tile pairs: PE order is Spair(0) Spair(1)
            # AV(0) AV(1) Spair(2) AV(2) AV(3) ... so AV(t) reaches the PE
            # queue head only after the next pair's scores -- exp(pair) done.
            NP = (NT + 1) // 2
            sts = {0: emit_pair_scores(0)}
            es = {}
            for m in range(NP):
                if m + 1 < NP:
                    sts[m + 1] = emit_pair_scores(m + 1)
                es[m] = emit_exp(m, sts.pop(m))
                for t in (2 * m, 2 * m + 1):
                    if t < NT:
                        emit_norm(t, emit_av(t, es[m]))
                        emit_outs(t)
                es.pop(m)

    nc.compile()
    _NC_CACHE = nc
    return nc


def _prep_inputs(X):
    X = np.ascontiguousarray(np.asarray(X, dtype=np.float32))
    in_maps = []
    for c in range(N_CORES):
        b, half = c // 2, c % 2
        r_in0 = 0 if half == 0 else H - SH_ROWS_IN          # 0 or 28
        shard = X[b, r_in0 : r_in0 + SH_ROWS_IN]            # [36, 64, 256]
        shard_bf = shard.astype(BF16)
        ext = np.empty((SH_ROWS_IN, W, C + 1), dtype=BF16)
        ext[:, :, :C] = shard_bf
        ext[:, :, C] = np.asarray(1.0, dtype=BF16)

        blob = np.zeros((128, BLOB_TOT), dtype=BF16)
        blob[:NQ, :BLOB_CB] = _CBIAS
        for ti, w0 in enumerate(W0S):
            o = BLOB_CB + ti * PANEL_ELEMS
            sl = shard_bf[:, w0 : w0 + KT, :]               # [36, 16, 256]
            slT = sl.reshape(PAN, C).T.reshape(2, 128, PAN)  # [k, 128, 576]
            blob[:, o : o + 2 * PAN] = (
                slT.transpose(1, 0, 2).reshape(128, 2 * PAN))
            o += 2 * PAN
            for ri, r0 in enumerate(R0S):
                blk = ext[r0 : r0 + KT, w0 : w0 + KT, :]    # [16, 16, 257]
                bj = blk.reshape(2, 128, C + 1)             # [j, 128, 257]
                blob[:, o + ri * 2 * (C + 1) : o + (ri + 1) * 2 * (C + 1)] = (
                    bj.transpose(1, 0, 2).reshape(128, 2 * (C + 1)))
        in_maps.append({"blob": blob})
    return in_maps


def _gather_simple(results):
    full = np.empty((B, HO, WO, C), dtype=np.float32)
    for c in range(N_CORES):
        b, half = c // 2, c % 2
        o = results[c]["out"].astype(np.float32)            # [100, 18, 256]
        o4 = o.reshape(QT, QT, NT, C)                       # [qr,qc,pos,C]
        rbase = 0 if half == 0 else H - SH_ROWS_IN          # shard row offset
        for pos, t in enumerate(TILE_ORDER):
            w0, r0 = W0S[t // 3], R0S[t % 3]
            rows = rbase + r0
            full[b, rows : rows + QT, w0 : w0 + QT, :] = o4[:, :, pos]
    return full


def _run(X, trace=False, **kw):
    from concourse.bass_utils import run_bass_kernel_spmd

    nc = _build_bass()
    in_maps = _prep_inputs(X)
    res = run_bass_kernel_spmd(nc, in_maps, core_ids=list(range(N_CORES)),
                               trace=trace, **kw)
    return res


def kernel(X):
    res = _run(X, trace=False)
    return _gather_simple(res.results)


# revision 70
# speedup vs baseline: 1.0065x; 1.0065x over previous
"""Trainium2 Bass kernel: 7x7 local window attention (ConvNDAttention).

Input  X: [4, 64, 64, 256] fp32 (channel-last).
Output:   [4, 58, 58, 256] fp32.

For each output position (b, r, w): 7x7 input window rows r..r+6, cols
w..w+6; query = center cell (r+3, w+3); keys/values = the other 48 cells.
out = softmax(q . K / 16) @ K.

Sharding: 8 cores = 4 batches x 2 row-halves (30 output rows each, 2-row
overlap).  Per core, 18 tiles of 10x10 queries; each tile's keys are a 16x16
input patch (256 keys, 2 chunks of 128).

Design notes (cost-model driven; baseline 48.2us -> ~20.3us per core):
  - Each DMA costs ~625ns on the serialized HWDGE queue + ~650ns issue +
    ~900ns completion-sem, and all transfers serialize on the DMA engines
    at ~360 B/ns, so the whole input ships as ONE packed DRAM tensor
    ("blob", 4.2MB bf16) split into 7 paired per-panel DMAs: cut c carries
    vs(panel c-1) + xp(panel c), so compute on panel c starts as soon as
    its cut lands while panel c-1's AV data arrives just in time.
  - The window-validity mask is folded into the scores PSUM accumulation
    as a 5th matmul per (tile, key-chunk): PSUM[k,q] += cb[q,j,k] with
    rhs = I_100 (contraction over q).  Invalid pairs get -1e4 (exp
    underflows to exactly 0), valid pairs exact +0 -- no separate
    mask multiply, no extra cross-engine hop, no precision impact.
  - Two tiles' score PSUM share one 2KB bank ([128,2,2,100] fp32), so ONE
    ACT exp instruction converts both (amortizes ~143ns PSUM access cost).
  - PE queue is in-order: scores of pair m+1 are emitted before AV of pair
    m, so the PE never head-of-line blocks waiting for exp.
  - Normalize (out = AV[:, :256] * 1/AV[:, 256]) runs mostly on DVE, every
    4th tile on ACT (Pool/GPSIMD cannot read PSUM on real hw).
  - Output accumulates in SBUF bf16 and leaves in 4 chunks: early chunks
    via Pool/SWDGE (latency-tolerant), the tiny last chunk via SP/HWDGE.

Host-side layouts per core (bf16), packed into blob [128, 16520]:
  cb [100, 356]        mask-bias weights (-1e4/0) + identity I_100.
  xp [128, 2, 576]/panel  channel-major column panels (panel ti covers
      input cols W0S[ti]..+15, all 36 rows; free index s = row*16 + col;
      chunk k = channels 128k..+128 on partitions).
  vs [128, 3, 2, 257]/panel  spatial-major values + ones column per tile
      (partition = key-in-chunk = (row%8)*16+col, chunk j = key rows
      r0+8j..+8; the ones column turns AV col 256 into softmax row sums).
  out [100, 18, 256]   bf16; host scatters tiles into the full output.
"""

import numpy as np
import ml_dtypes

import concourse.bass as bass
import concourse.bacc as bacc
import concourse.mybir as mybir
import concourse.tile as tile

BF16 = ml_dtypes.bfloat16

# ---------------- geometry (hardcoded for X [4,64,64,256]) ----------------
B, H, W, C = 4, 64, 64, 256
HO, WO = H - 6, W - 6          # 58 x 58 output
N_CORES = 8
SH_ROWS_IN = 36                # input rows per shard
SH_ROWS_OUT = 30               # output rows per shard
R0S = [0, 10, 20]              # tile row origins (shard-local output rows)
W0S = [0, 10, 20, 30, 40, 48]  # tile col origins
QT = 10                        # query tile side
KT = 16                        # key patch side
NQ = QT * QT                   # 100 queries per tile
NT = len(W0S) * len(R0S)       # 18 tiles
PAN = SH_ROWS_IN * KT          # 576 panel spatial positions
BLOB_CB = 2 * 128 + NQ         # 356: bias weights + identity
PANEL_ELEMS = 2 * PAN + 3 * 2 * (C + 1)     # 2694: xp panel + 3 vs tiles
BLOB_TOT = BLOB_CB + len(W0S) * PANEL_ELEMS  # 16520 bf16 elems/partition
# tile processing order (identity: reorderings tested slower -- the stream
# is input-DMA-paced, so deferring tiles only starves the early pipeline);
# the device output tensor is indexed by POSITION in this order
TILE_ORDER = list(range(18))


def _build_bias_consts():
    """cb [100, 2*128 + 100] bf16: score-bias weights + identity.

    cb[q, j*128+k] = -1e4 where key (k, chunk j) is invalid for query q
    (exp underflows to exactly 0), 0 where valid (exact no-op).
    cb[q, 256:356] = I_100.  Added into the scores PSUM group via
    matmul(lhsT=cb[:, j], rhs=I): out[k, q] += cb[q, j*128+k] * I[q, q].
    """
    p = np.arange(128)
    kh = p[None, None, :] // KT + 8 * np.arange(2)[None, :, None]  # [1,2,128]
    kw = (p % KT)[None, None, :]
    q = np.arange(NQ)[:, None, None]
    qh, qw = q // QT, q % QT
    dy, dx = kh - qh, kw - qw
    valid = ((dy >= 0) & (dy <= 6) & (dx >= 0) & (dx <= 6)
             & ~((dy == 3) & (dx == 3)))                  # [100, 2, 128]
    cb = np.zeros((NQ, 2 * 128 + NQ), dtype=np.float32)
    cb[:, : 2 * 128] = np.where(valid, 0.0, -1e4).reshape(NQ, 256)
    cb[:, 2 * 128 :] = np.eye(NQ)
    return cb.astype(BF16)


_CBIAS = _build_bias_consts()

_NC_CACHE = None


def _build_bass():
    global _NC_CACHE
    if _NC_CACHE is not None:
        return _NC_CACHE
    nc = bacc.Bacc("TRN2")
    dt = mybir.dt

    blob = nc.dram_tensor("blob", [128, BLOB_TOT], dt.bfloat16,
                          kind="ExternalInput")
    out = nc.dram_tensor("out", [NQ, NT, C], dt.bfloat16,
                         kind="ExternalOutput")

    with tile.TileContext(nc) as tc:
        with (
            tc.tile_pool(name="const", bufs=1) as const_pool,
            tc.tile_pool(name="ework", bufs=6) as e_pool,
            tc.tile_pool(name="rwork", bufs=6) as r_pool,
            tc.tile_pool(name="owork", bufs=2) as o_pool,
            tc.tile_pool(name="ps_s", bufs=2, space="PSUM") as ps_s,
            tc.tile_pool(name="ps_av", bufs=6, space="PSUM") as ps_av,
            # ^ ps_s tiles hold TWO tiles' scores in one 1600B bank so one
            #   ACT exp instruction covers both (fixed access cost amortized)
        ):
            blob_sb = const_pool.tile([128, BLOB_TOT], dt.bfloat16, tag="blob")
            # paired per-panel loads: cut c delivers vs(panel c-1) + xp(panel
            # c) so S(panel c) / AV(panel c-1) start as soon as it lands
            # (finer tail cuts tested slower: extra per-DMA dge latency
            # punches gaps into the otherwise gapless transfer stream)
            cuts = ([0, BLOB_CB + 2 * PAN]
                    + [BLOB_CB + 2 * PAN + (t + 1) * PANEL_ELEMS
                       for t in range(len(W0S) - 1)]
                    + [BLOB_TOT])
            for a, b in zip(cuts[:-1], cuts[1:]):
                nc.sync.dma_start(out=blob_sb[:, a:b], in_=blob[:, a:b])

            cb_v = blob_sb[:NQ, : 2 * 128].rearrange("q (a k) -> q a k", k=128)
            iden = blob_sb[:NQ, 2 * 128 : BLOB_CB]

            def xp_pan(ti):
                o = BLOB_CB + ti * PANEL_ELEMS
                return blob_sb[:, o : o + 2 * PAN].rearrange(
                    "p (k s) -> p k s", s=PAN)

            def vs_pan(ti):
                o = BLOB_CB + ti * PANEL_ELEMS + 2 * PAN
                return blob_sb[:, o : o + 6 * (C + 1)].rearrange(
                    "p (r j c) -> p r j c", j=2, c=C + 1)

            o_all = o_pool.tile([NQ, NT, C], dt.bfloat16, tag="oall")
            # output chunks aligned to input-panel arrival; the last chunks
            # are tiny so the final DMA latency is minimal
            OCUTS = [0, 6, 12, 15, NT]

            tiles_nat = [(ti, w0, ri, r0)
                         for ti, w0 in enumerate(W0S)
                         for ri, r0 in enumerate(R0S)]
            tiles = [tiles_nat[t] for t in TILE_ORDER]

            def emit_scores(t, st):
                # writes scores into half t%2 of the pair tile st
                ti, w0, ri, r0 = tiles[t]
                xpt = xp_pan(ti)
                pan = [xpt[:, k, :] for k in range(2)]
                pan_v = [p.rearrange("p (h w) -> p h w", w=KT) for p in pan]
                for j in range(2):
                    for k in range(2):
                        keys = pan[k][:, (r0 + 8 * j) * KT :
                                      (r0 + 8 * j) * KT + 128]
                        qrys = pan_v[k][:, r0 + 3 : r0 + 3 + QT, 3 : 3 + QT]
                        nc.tensor.matmul(st[:, t % 2, j, :], lhsT=keys,
                                         rhs=qrys, start=(k == 0), stop=False)
                    # window-validity bias: st[k, q] += cb[q, j, k] (-1e4 on
                    # invalid pairs -> exp underflows to 0; exact 0 on valid)
                    nc.tensor.matmul(st[:, t % 2, j, :], lhsT=cb_v[:, j, :],
                                     rhs=iden, start=False, stop=True)

            def emit_pair_scores(m):
                st = ps_s.tile([128, 2, 2, NQ], dt.float32, tag="st")
                emit_scores(2 * m, st)
                if 2 * m + 1 < NT:
                    emit_scores(2 * m + 1, st)
                return st

            def emit_exp(m, st):
                # E = exp(S/16) -> bf16 SBUF, both tiles of the pair at once
                e = e_pool.tile([128, 2, 2, NQ], dt.bfloat16, tag="e")
                nc.scalar.activation(e[:, :, :, :], st[:, :, :, :],
                                     mybir.ActivationFunctionType.Exp,
                                     scale=1.0 / 16.0)
                return e

            def emit_av(t, e):
                ti, w0, ri, r0 = tiles[t]
                av = ps_av.tile([NQ, C + 1], dt.float32, tag="av")
                for j in range(2):
                    nc.tensor.matmul(av[:, :], lhsT=e[:, t % 2, j, :],
                                     rhs=vs_pan(ti)[:, ri, j, :],
                                     start=(j == 0), stop=(j == 1))
                return av

            def emit_norm(t, av):
                rinv = r_pool.tile([NQ, 1], dt.float32, tag="rinv")
                nc.vector.reciprocal(rinv[:, :], av[:, C : C + 1])
                dst = o_all[:, t, :]
                # mostly DVE; ACT (exp-laden) takes every 4th to balance
                # (Pool/GPSIMD cannot read PSUM on hw); t=16 on ACT so the
                # tail norms run on two engines in parallel
                if t % 4 == 2 or t == 16:
                    nc.scalar.mul(dst, av[:, 0:C], rinv[:, :])
                else:
                    nc.vector.tensor_scalar_mul(dst, av[:, 0:C], rinv[:, :])

            def emit_outs(t):
                for ci, (a, b) in enumerate(zip(OCUTS[:-1], OCUTS[1:])):
                    if t == b - 1:   # positions a..b-1 normalized by now
                        # big early chunks via Pool/SWDGE (latency-tolerant);
                        # tail chunks split across ACT and SP HWDGE queues
                        # so their issues run in parallel
                        eng = nc.sync if b == NT else nc.gpsimd
                        eng.dma_start(out=out[:, a:b, :],
                                      in_=o_all[:, a:b, :])

            # software pipeline in tile pairs: PE order is Spair(0) Spair(1)
            # AV(0) AV(1) Spair(2) AV(2) AV(3) ... so AV(t) reaches the PE
            # queue head only after the next pair's scores -- exp(pair) done.
            NP = (NT + 1) // 2
            sts = {0: emit_pair_scores(0)}
            es = {}
            for m in range(NP):
                if m + 1 < NP:
                    sts[m + 1] = emit_pair_scores(m + 1)
                es[m] = emit_exp(m, sts.pop(m))
                for t in (2 * m, 2 * m + 1):
                    if t < NT:
                        emit_norm(t, emit_av(t, es[m]))
                        emit_outs(t)
                es.pop(m)

    nc.compile()
    _NC_CACHE = nc
    return nc


def _prep_inputs(X):
    X = np.ascontiguousarray(np.asarray(X, dtype=np.float32))
    in_maps = []
    for c in range(N_CORES):
        b, half = c // 2, c % 2
        r_in0 = 0 if half == 0 else H - SH_ROWS_IN          # 0 or 28
        shard = X[b, r_in0 : r_in0 + SH_ROWS_IN]            # [36, 64, 256]
        shard_bf = shard.astype(BF16)
        ext = np.empty((SH_ROWS_IN, W, C + 1), dtype=BF16)
        ext[:, :, :C] = shard_bf
        ext[:, :, C] = np.asarray(1.0, dtype=BF16)

        blob = np.zeros((128, BLOB_TOT), dtype=BF16)
        blob[:NQ, :BLOB_CB] = _CBIAS
        for ti, w0 in enumerate(W0S):
            o = BLOB_CB + ti * PANEL_ELEMS
            sl = shard_bf[:, w0 : w0 + KT, :]               # [36, 16, 256]
            slT = sl.reshape(PAN, C).T.reshape(2, 128, PAN)  # [k, 128, 576]
            blob[:, o : o + 2 * PAN] = (
                slT.transpose(1, 0, 2).reshape(128, 2 * PAN))
            o += 2 * PAN
            for ri, r0 in enumerate(R0S):
                blk = ext[r0 : r0 + KT, w0 : w0 + KT, :]    # [16, 16, 257]
                bj = blk.reshape(2, 128, C + 1)             # [j, 128, 257]
                blob[:, o + ri * 2 * (C + 1) : o + (ri + 1) * 2 * (C + 1)] = (
                    bj.transpose(1, 0, 2).reshape(128, 2 * (C + 1)))
        in_maps.append({"blob": blob})
    return in_maps


def _gather_simple(results):
    full = np.empty((B, HO, WO, C), dtype=np.float32)
    for c in range(N_CORES):
        b, half = c // 2, c % 2
        o = results[c]["out"].astype(np.float32)            # [100, 18, 256]
        o4 = o.reshape(QT, QT, NT, C)                       # [qr,qc,pos,C]
        rbase = 0 if half == 0 else H - SH_ROWS_IN          # shard row offset
        for pos, t in enumerate(TILE_ORDER):
            w0, r0 = W0S[t // 3], R0S[t % 3]
            rows = rbase + r0
            full[b, rows : rows + QT, w0 : w0 + QT, :] = o4[:, :, pos]
    return full


def _run(X, trace=False, **kw):
    from concourse.bass_utils import run_bass_kernel_spmd

    nc = _build_bass()
    in_maps = _prep_inputs(X)
    res = run_bass_kernel_spmd(nc, in_maps, core_ids=list(range(N_CORES)),
                               trace=trace, **kw)
    return res


def kernel(X):
    res = _run(X, trace=False)
    return _gather_simple(res.results)
